# revision 1
# baseline (speedup 1.0000x reference)
"""Bass/Trainium2 kernel for nn_DTSP (GNN message passing, 8 graphs x K100).

Sharding: data-parallel, 1 graph per NeuronCore (8 cores). Each core runs the
full 32-step message-passing recurrence for its graph; the only cross-device
step is the host-side gather of the 8 per-graph vote scalars.

On-chip layout (per core):
  - Edge tensors are feature-major with the 4950 (padded 5120) edges split in
    two halves of 2560, stacked on the partition axis -> [128, 2560] tiles so
    elementwise ops run at full 128-lane width.
  - All MLP/LSTM matmuls contract over features (K=64/128 on partitions).
    The two halves run concurrently on the PE via disjoint 64-row/64-col
    tile quadrants (tile_position auto-derived from AP base partitions).
  - EV aggregation (edges->vertices) streams row-major msg chunks as the
    stationary operand against a host-prepared chunked EV layout; the
    vertex->edge scatter streams a host-pretransposed EV^T (with an extra
    row carrying EV row-degrees, which folds the vm_b2 bias in).
  - All constant biases are folded into activation bias operands or extra
    matmul contraction rows on the host; em_b2 folds into the xV aggregation
    via a rank-1 (em_b2 x vertex-degree) matmul appended to the accum group.
"""

import os
import numpy as np

B = 8
V = 100
E = 4950
DIM = 64
NUM_MP = int(os.environ.get("DTSP_NUM_MP", "32"))
PHASE = int(os.environ.get("DTSP_PHASE", "99"))
LOOP_R = int(os.environ.get("DTSP_LOOP_R", "0"))
EPAD = 5120
H = EPAD // 2            # 2560 columns per half
CH = 512                 # psum chunk width
NCHUNK = H // CH         # 5
NSUB = H // 128          # 20 row-major 128-edge subchunks per half
NSUB_T = 2 * NSUB        # 40 subchunks total

_CACHE = {}


def _build_bass(num_mp=None, loop_r=None, phase=None):
    global NUM_MP, LOOP_R, PHASE
    if num_mp is not None:
        NUM_MP = num_mp
    if loop_r is not None:
        LOOP_R = loop_r
    if phase is not None:
        PHASE = phase
    import concourse.bacc as bacc
    import concourse.tile as tile
    from concourse import mybir

    f32 = mybir.dt.float32
    AF = mybir.ActivationFunctionType
    OP = mybir.AluOpType

    nc = bacc.Bacc("TRN2", target_bir_lowering=False, debug=False)

    # ---- DRAM I/O ----
    d_wfc = nc.dram_tensor("wfc", [128, H], f32, kind="ExternalInput")
    d_evr = nc.dram_tensor("evr", [128, NSUB_T * V], f32, kind="ExternalInput")
    d_evtx = nc.dram_tensor("evtx", [V + 1, EPAD], f32, kind="ExternalInput")
    d_wmlp = nc.dram_tensor("wmlp", [128, 8 * 64], f32, kind="ExternalInput")
    d_wcat = nc.dram_tensor("wcat", [128, 3 * 256], f32, kind="ExternalInput")
    d_winit = nc.dram_tensor("winit", [128, 120], f32, kind="ExternalInput")
    d_bias = nc.dram_tensor("bias", [128, 24], f32, kind="ExternalInput")
    d_vmisc = nc.dram_tensor("vmisc", [128, 384], f32, kind="ExternalInput")
    d_vh0 = nc.dram_tensor("vh0", [128, 128], f32, kind="ExternalInput")
    d_mlpx0 = nc.dram_tensor("mlpx0", [128, 64], f32, kind="ExternalInput")
    d_out = nc.dram_tensor("out", [1, 1], f32, kind="ExternalOutput")

    with tile.TileContext(nc) as tc:
        import contextlib
        ctx = contextlib.ExitStack()
        with ctx:
            st = ctx.enter_context(tc.tile_pool(name="state", bufs=1))
            ps = ctx.enter_context(tc.tile_pool(name="ps", bufs=7, space="PSUM"))
            psv = ctx.enter_context(tc.tile_pool(name="psv", bufs=1, space="PSUM"))
            psa = psv
            psr = psv

            # ---- persistent SBUF tiles ----
            evr = st.tile([128, NSUB_T * V], f32, tag="evr")
            evtx = st.tile([V + 1, EPAD], f32, tag="evtx")
            wmlp = st.tile([128, 8 * 64], f32, tag="wmlp")
            wcat = st.tile([128, 3 * 256], f32, tag="wcat")
            winit = st.tile([128, 120], f32, tag="winit")
            bias = st.tile([128, 24], f32, tag="bias")
            vmisc = st.tile([128, 384], f32, tag="vmisc")

            xh0 = st.tile([128, H], f32, tag="xh0")   # [xE_h0 ; E_h0]
            xh1 = st.tile([128, H], f32, tag="xh1")   # [E_h1 ; xE_h1]
            cE = st.tile([128, H], f32, tag="cE")     # c state (h0 on 64:128)
            xE = st.tile([128, H], f32, tag="xE")     # xET evac (h0 on 0:64)
            h1sb = st.tile([128, H], f32, tag="h1")
            h2sb = st.tile([128, H], f32, tag="h2")
            msg = st.tile([128, H], f32, tag="msg")   # row-major msg chunks
            s_i = st.tile([128, H], f32, tag="si")
            s_f = st.tile([128, H], f32, tag="sf")
            t_g = st.tile([128, H], f32, tag="tg")
            s_o = st.tile([128, H], f32, tag="so")
            tcE = st.tile([128, H], f32, tag="tc")

            # V-side small tiles
            xh_v = st.tile([128, 128], f32, tag="xhv")   # [xVT ; VT]
            cV = st.tile([128, 128], f32, tag="cv")
            sv = [st.tile([128, 128], f32, tag=f"sv{g}", name=f"sv{g}") for g in range(4)]
            tcV = st.tile([128, 128], f32, tag="tcv")
            h1v = st.tile([128, 128], f32, tag="h1v")
            h2v = st.tile([128, 128], f32, tag="h2v")
            mlpx = st.tile([128, 64], f32, tag="mlpx")
            rsum = st.tile([128, 1], f32, tag="rsum")
            outsb = st.tile([1, 4], f32, tag="outsb")

            # ---- weight views ----
            def wm(i):  # [128, 64] doubled weight i from wmlp
                return wmlp[:, i * 64:(i + 1) * 64]

            em_w0d, em_w1d, em_w2d = wm(0), wm(1), wm(2)
            vm_w0d, vm_w1d, vm_w2d = wm(3), wm(4), wm(5)
            vt_w0d, vt_w1d = wm(6), wm(7)

            e_wcat0 = wcat[:, 0:256]
            e_wcat1 = wcat[:, 256:512]
            v_wcat = wcat[:, 512:768]

            iw0d = winit[:, 0:8]
            iw1d = winit[:, 8:24]
            iw2d = winit[:, 24:56]
            iw3d = winit[:, 56:120]

            # bias columns (see host packing in kernel())
            def bcol(j):
                return bias[:, j:j + 1]

            ib0d, ib1d, ib2d, ib3d = bcol(0), bcol(1), bcol(2), bcol(3)
            em_b0d, em_b1d = bcol(4), bcol(5)
            vm_b0d, vm_b1d = bcol(6), bcol(7)
            vt_b0d, vt_b1d = bcol(8), bcol(9)
            be = [bcol(10 + g) for g in range(4)]     # E-LSTM gate biases
            bv = [bcol(14 + g) for g in range(4)]     # V-LSTM gate biases
            w2scaled = bcol(18)                        # [vt_w2;vt_w2]/E
            vt_b2d = bcol(19)

            emb2row = vmisc[:, 0:64]                   # row 0: em_b2
            degv = vmisc[:, 256:384]                   # row 0: EV col-degrees

            def cs(c):
                return slice(c * CH, (c + 1) * CH)

            GATE_FUNC = [AF.Sigmoid, AF.Sigmoid, AF.Tanh, AF.Sigmoid]
            GATE_DST = [s_i, s_f, t_g, s_o]

            # ================= INIT =================
            nc.sync.dma_start(out=winit[:], in_=d_winit[:])
            nc.sync.dma_start(out=bias[:], in_=d_bias[:])
            nc.sync.dma_start(out=vmisc[:], in_=d_vmisc[:])
            nc.sync.dma_start(out=wmlp[:], in_=d_wmlp[:])
            nc.sync.dma_start(out=wcat[:], in_=d_wcat[:])
            nc.sync.dma_start(out=xh_v[:], in_=d_vh0[:])
            nc.sync.dma_start(out=mlpx[:], in_=d_mlpx0[:])
            nc.sync.dma_start(out=evr[:], in_=d_evr[:])
            nc.sync.dma_start(out=evtx[:], in_=d_evtx[:])

            nc.vector.memset(cE[:], 0.0)
            nc.vector.memset(cV[:], 0.0)

            # init 4-layer MLP on [W,C] -> E0 (into xh0[64:128], xh1[0:64])
            wfc = st.tile([128, H], f32, tag="wfc")
            nc.sync.dma_start(out=wfc[:], in_=d_wfc[:])
            li1, li2, li3 = h1sb, h2sb, msg
            for c in range(NCHUNK):
                p = ps.tile([128, CH], f32, tag="mm")
                nc.tensor.matmul(p[0:8, :], iw0d[0:2, :], wfc[0:2, cs(c)])
                nc.tensor.matmul(p[32:40, :], iw0d[64:66, :], wfc[64:66, cs(c)])
                nc.scalar.activation(li1[0:8, cs(c)], p[0:8, :], AF.Relu, bias=ib0d[0:8])
                nc.scalar.activation(li1[32:40, cs(c)], p[32:40, :], AF.Relu, bias=ib0d[32:40])
            for c in range(NCHUNK):
                p = ps.tile([128, CH], f32, tag="mm")
                nc.tensor.matmul(p[0:16, :], iw1d[0:8, :], li1[0:8, cs(c)])
                nc.tensor.matmul(p[32:48, :], iw1d[32:40, :], li1[32:40, cs(c)])
                nc.scalar.activation(li2[0:16, cs(c)], p[0:16, :], AF.Relu, bias=ib1d[0:16])
                nc.scalar.activation(li2[32:48, cs(c)], p[32:48, :], AF.Relu, bias=ib1d[32:48])
            for c in range(NCHUNK):
                p = ps.tile([128, CH], f32, tag="mm")
                nc.tensor.matmul(p[0:32, :], iw2d[0:16, :], li2[0:16, cs(c)])
                nc.tensor.matmul(p[32:64, :], iw2d[32:48, :], li2[32:48, cs(c)])
                nc.scalar.activation(li3[0:32, cs(c)], p[0:32, :], AF.Relu, bias=ib2d[0:32])
                nc.scalar.activation(li3[32:64, cs(c)], p[32:64, :], AF.Relu, bias=ib2d[32:64])
            for c in range(NCHUNK):
                p = ps.tile([128, CH], f32, tag="mm")
                nc.tensor.matmul(p[64:128, :], iw3d[0:32, :], li3[0:32, cs(c)])
                nc.tensor.matmul(p[0:64, :], iw3d[32:64, :], li3[32:64, cs(c)])
                nc.scalar.activation(s_o[64:128, cs(c)], p[64:128, :], AF.Identity, bias=ib3d[64:128])
                nc.scalar.activation(s_o[0:64, cs(c)], p[0:64, :], AF.Identity, bias=ib3d[0:64])
            nc.gpsimd.tensor_copy(xh0[64:128, :], s_o[64:128, :])
            nc.gpsimd.tensor_copy(xh1[0:64, :], s_o[0:64, :])

            # ================= MP STEPS =================
            def emit_step():
                # --- E-MLP layer 1: h1 = relu(W0^T E + b0) ---
                for c in range(NCHUNK):
                    p = ps.tile([128, CH], f32, tag="mm")
                    nc.tensor.matmul(p[0:64, :], em_w0d[64:128, :], s_o[64:128, cs(c)])
                    nc.tensor.matmul(p[64:128, :], em_w0d[0:64, :], s_o[0:64, cs(c)])
                    nc.scalar.activation(h1sb[:, cs(c)], p[:, :], AF.Relu, bias=em_b0d)
                # --- E-MLP layer 2 ---
                for c in range(NCHUNK):
                    p = ps.tile([128, CH], f32, tag="mm")
                    nc.tensor.matmul(p[0:64, :], em_w1d[0:64, :], h1sb[0:64, cs(c)])
                    nc.tensor.matmul(p[64:128, :], em_w1d[64:128, :], h1sb[64:128, cs(c)])
                    nc.scalar.activation(h2sb[:, cs(c)], p[:, :], AF.Relu, bias=em_b1d)
                if PHASE < 2:
                    return
                # --- E-MLP layer 3 (row-major msg chunks) + aggregation ---
                # NOTE: matmuls from different row-groups must not share a
                # psum bank (HW fault) -> 4 same-half chunks per [128, 256] tile
                aggp = psa.tile([64, 128], f32, tag="vg")
                for blk in range(NSUB_T // 4):
                    mp = ps.tile([128, 256], f32, tag="mm")
                    for k in range(4):
                        m = blk * 4 + k
                        half, i = divmod(m, NSUB)
                        hsl = slice(half * 64, half * 64 + 64)
                        nc.tensor.matmul(
                            mp[:, k * 64:(k + 1) * 64],
                            h2sb[hsl, i * 128:(i + 1) * 128],
                            em_w2d[hsl, :],
                        )
                    nc.vector.tensor_copy(msg[:, blk * 256:(blk + 1) * 256], mp[:, :])
                # separate pass so agg matmuls never stall on the msg evacs
                for m in range(NSUB_T):
                    nc.tensor.matmul(
                        aggp[:, 0:V],
                        msg[:, m * 64:(m + 1) * 64],
                        evr[:, m * V:(m + 1) * V],
                        start=(m == 0),
                        stop=False,
                    )
                # += deg (x) em_b2  (xV bias from the folded msg-layer bias)
                nc.tensor.matmul(aggp[:, 0:V], emb2row[0:1, :], degv[0:1, 0:V],
                                 start=False, stop=True)
                if PHASE < 3:
                    return
                # --- V side ---
                nc.scalar.activation(xh_v[0:64, 0:V], aggp[:, 0:V], AF.Copy)
                for g in range(4):
                    vp = psv.tile([128, 128], f32, tag="vg")
                    nc.tensor.matmul(vp[64:128, 0:V], v_wcat[:, g * 64:(g + 1) * 64],
                                     xh_v[:, 0:V])
                    nc.scalar.activation(sv[g][64:128, 0:V], vp[64:128, 0:V],
                                         GATE_FUNC[g], bias=bv[g][64:128])
                nc.vector.tensor_tensor(cV[64:128, 0:V], cV[64:128, 0:V], sv[1][64:128, 0:V], OP.mult)
                nc.vector.tensor_tensor(sv[2][64:128, 0:V], sv[0][64:128, 0:V], sv[2][64:128, 0:V], OP.mult)
                nc.vector.tensor_tensor(cV[64:128, 0:V], cV[64:128, 0:V], sv[2][64:128, 0:V], OP.add)
                nc.scalar.activation(tcV[64:128, 0:V], cV[64:128, 0:V], AF.Tanh)
                nc.vector.tensor_tensor(xh_v[64:128, 0:V], sv[3][64:128, 0:V], tcV[64:128, 0:V], OP.mult)
                # --- mlpV ---
                vp = psv.tile([128, 128], f32, tag="vg")
                nc.tensor.matmul(vp[0:64, 0:V], vm_w0d[64:128, :], xh_v[64:128, 0:V])
                nc.scalar.activation(h1v[0:64, 0:V], vp[0:64, 0:V], AF.Relu, bias=vm_b0d[0:64])
                vp = psv.tile([128, 128], f32, tag="vg")
                nc.tensor.matmul(vp[0:64, 0:V], vm_w1d[0:64, :], h1v[0:64, 0:V])
                nc.scalar.activation(h2v[0:64, 0:V], vp[0:64, 0:V], AF.Relu, bias=vm_b1d[0:64])
                pr = psr.tile([128, 64], f32, tag="vg")
                nc.tensor.matmul(pr[0:V, :], h2v[0:64, 0:V], vm_w2d[0:64, :])
                nc.scalar.activation(mlpx[0:V, :], pr[0:V, :], AF.Copy)
                if PHASE < 4:
                    return
                # --- xET = mlpx^T @ EVT (+ rowdeg*vm_b2 via row 100) ---
                for c in range(NCHUNK):
                    px = ps.tile([128, CH], f32, tag="mm")
                    nc.tensor.matmul(px[0:64, :], mlpx[0:V + 1, :], evtx[0:V + 1, cs(c)])
                    nc.tensor.matmul(px[64:128, :], mlpx[0:V + 1, :],
                                     evtx[0:V + 1, H + c * CH:H + (c + 1) * CH])
                    nc.vector.tensor_copy(xh0[0:64, cs(c)], px[0:64, :])
                    nc.vector.tensor_copy(xh1[64:128, cs(c)], px[64:128, :])
                if PHASE < 5:
                    return
                # --- E-LSTM gates ---
                for c in range(NCHUNK):
                    for g in range(4):
                        gp = ps.tile([128, CH], f32, tag="mm")
                        nc.tensor.matmul(gp[64:128, :], e_wcat0[:, g * 64:(g + 1) * 64],
                                         xh0[:, cs(c)])
                        nc.tensor.matmul(gp[0:64, :], e_wcat1[:, g * 64:(g + 1) * 64],
                                         xh1[:, cs(c)])
                        nc.scalar.activation(GATE_DST[g][:, cs(c)], gp[:, :],
                                             GATE_FUNC[g], bias=be[g])
                if PHASE < 6:
                    return
                # --- E-LSTM state update ---
                for cc in range(4):
                    sl = slice(cc * (H // 4), (cc + 1) * (H // 4))
                    nc.gpsimd.tensor_tensor(t_g[:, sl], s_i[:, sl], t_g[:, sl], OP.mult)
                    nc.vector.tensor_tensor(cE[:, sl], cE[:, sl], s_f[:, sl], OP.mult)
                    nc.vector.tensor_tensor(cE[:, sl], cE[:, sl], t_g[:, sl], OP.add)
                    nc.scalar.activation(tcE[:, sl], cE[:, sl], AF.Tanh)
                    nc.vector.tensor_tensor(s_o[:, sl], s_o[:, sl], tcE[:, sl], OP.mult)
                    nc.gpsimd.tensor_copy(xh0[64:128, sl], s_o[64:128, sl])
                    nc.gpsimd.tensor_copy(xh1[0:64, sl], s_o[0:64, sl])

            if LOOP_R:
                with tc.For_i(0, LOOP_R, 1):
                    emit_step()
            else:
                for _t in range(NUM_MP):
                    emit_step()

            # ================= VOTE =================
            for c in range(NCHUNK):
                p = ps.tile([128, CH], f32, tag="mm")
                nc.tensor.matmul(p[0:64, :], vt_w0d[64:128, :], s_o[64:128, cs(c)])
                nc.tensor.matmul(p[64:128, :], vt_w0d[0:64, :], s_o[0:64, cs(c)])
                nc.vector.tensor_scalar(h1sb[:, cs(c)], p[:, :], vt_b0d, 0.0, OP.add, OP.max)
            for c in range(NCHUNK):
                p = ps.tile([128, CH], f32, tag="mm")
                nc.tensor.matmul(p[0:64, :], vt_w1d[0:64, :], h1sb[0:64, cs(c)])
                nc.tensor.matmul(p[64:128, :], vt_w1d[64:128, :], h1sb[64:128, cs(c)])
                nc.vector.tensor_scalar(h2sb[:, cs(c)], p[:, :], vt_b1d, 0.0, OP.add, OP.max)
            # half0 = first 2560 padded edges (all real); half1 = 2390 real + pad
            nc.vector.reduce_sum(rsum[0:64, 0:1], h2sb[0:64, 0:H], axis=mybir.AxisListType.X)
            nc.vector.reduce_sum(rsum[64:128, 0:1], h2sb[64:128, 0:E - H], axis=mybir.AxisListType.X)
            vfin = psr.tile([128, 64], f32, tag="vg")
            nc.tensor.matmul(vfin[0:1, 0:1], rsum[:, 0:1], w2scaled[:, 0:1])
            nc.scalar.activation(outsb[0:1, 0:1], vfin[0:1, 0:1], AF.Identity, bias=vt_b2d[0:1])
            nc.sync.dma_start(out=d_out[:], in_=outsb[0:1, 0:1])

    nc.compile()
    return nc


def _prep_inputs(inputs):
    """Host-side: shard per graph + pack weights into the kernel's layouts."""
    gi = lambda k: np.asarray(inputs[k], dtype=np.float32)
    EV = gi("EV")
    Wfeat = gi("Wfeat").reshape(-1)
    C = gi("C").reshape(-1)

    # weights (shared across cores)
    def dbl(w):  # [64,64] -> [128,64] stacked twice
        return np.concatenate([w, w], axis=0).astype(np.float32)

    wmlp = np.concatenate(
        [dbl(gi("em_w0")), dbl(gi("em_w1")), dbl(gi("em_w2")),
         dbl(gi("vm_w0")), dbl(gi("vm_w1")), dbl(gi("vm_w2")),
         dbl(gi("vt_w0")), dbl(gi("vt_w1"))], axis=1)               # [128, 512]

    wih_e, whh_e = gi("wih_e"), gi("whh_e")
    wih_v, whh_v = gi("wih_v"), gi("whh_v")
    e_wcat0 = np.concatenate([wih_e, whh_e], axis=0)                 # [128, 256]
    e_wcat1 = np.concatenate([whh_e, wih_e], axis=0)
    v_wcat = np.concatenate([wih_v, whh_v], axis=0)
    wcat = np.concatenate([e_wcat0, e_wcat1, v_wcat], axis=1)        # [128, 768]

    winit = np.zeros((128, 120), np.float32)
    w0, w1, w2, w3 = gi("init_w0"), gi("init_w1"), gi("init_w2"), gi("init_w3")
    winit[0:2, 0:8] = w0; winit[64:66, 0:8] = w0
    winit[0:8, 8:24] = w1; winit[32:40, 8:24] = w1
    winit[0:16, 24:56] = w2; winit[32:48, 24:56] = w2
    winit[0:32, 56:120] = w3; winit[32:64, 56:120] = w3

    bias = np.zeros((128, 24), np.float32)
    b0, b1, b2, b3 = gi("init_b0"), gi("init_b1"), gi("init_b2"), gi("init_b3")
    bias[0:8, 0] = b0; bias[32:40, 0] = b0
    bias[0:16, 1] = b1; bias[32:48, 1] = b1
    bias[0:32, 2] = b2; bias[32:64, 2] = b2
    bias[:, 3] = np.tile(b3, 2)
    bias[:, 4] = np.tile(gi("em_b0"), 2)
    bias[:, 5] = np.tile(gi("em_b1"), 2)
    bias[0:64, 6] = gi("vm_b0")
    bias[0:64, 7] = gi("vm_b1")
    bias[:, 8] = np.tile(gi("vt_b0"), 2)
    bias[:, 9] = np.tile(gi("vt_b1"), 2)
    bih_e, bhh_e = gi("bih_e"), gi("bhh_e")
    bih_v, bhh_v = gi("bih_v"), gi("bhh_v")
    for g in range(4):
        bias[:, 10 + g] = np.tile((bih_e + bhh_e)[g * 64:(g + 1) * 64], 2)
        bias[64:128, 14 + g] = (bih_v + bhh_v)[g * 64:(g + 1) * 64]
    vt_w2 = gi("vt_w2").reshape(-1)
    bias[:, 18] = np.tile(vt_w2, 2) / float(E)
    bias[0, 19] = float(gi("vt_b2").reshape(-1)[0])

    em_b2 = gi("em_b2")

    vh0 = np.zeros((128, 128), np.float32)
    vinit = gi("v_init").reshape(-1) / np.sqrt(np.float32(DIM))
    vh0[64:128, 0:V] = np.repeat(vinit[:, None], V, axis=1)

    mlpx0 = np.zeros((128, 64), np.float32)
    mlpx0[V, :] = gi("vm_b2")

    per_core = []
    for b in range(B):
        ev = EV[b * E:(b + 1) * E, b * V:(b + 1) * V]
        evp = np.zeros((EPAD, V), np.float32)
        evp[:E, :] = ev
        evr = np.ascontiguousarray(
            evp.reshape(NSUB_T, 128, V).transpose(1, 0, 2).reshape(128, NSUB_T * V))
        evtx = np.zeros((V + 1, EPAD), np.float32)
        evtx[0:V, :] = evp.T
        evtx[V, :] = evp.sum(axis=1)                                # row degrees
        degv = ev.sum(axis=0)                                       # col degrees
        vmisc = np.zeros((128, 384), np.float32)
        vmisc[0, 0:64] = em_b2
        vmisc[0, 256:256 + V] = degv
        wfc = np.zeros((128, H), np.float32)
        w_b = Wfeat[b * E:(b + 1) * E]
        c_b = C[b * E:(b + 1) * E]
        wpad = np.zeros(EPAD, np.float32); wpad[:E] = w_b
        cpad = np.zeros(EPAD, np.float32); cpad[:E] = c_b
        wfc[0, :] = wpad[:H]; wfc[1, :] = cpad[:H]
        wfc[64, :] = wpad[H:]; wfc[65, :] = cpad[H:]
        per_core.append({
            "wfc": wfc, "evr": evr, "evtx": evtx, "wmlp": wmlp, "wcat": wcat,
            "winit": winit, "bias": bias, "vmisc": vmisc, "vh0": vh0,
            "mlpx0": mlpx0,
        })
    return per_core


def kernel(**inputs):
    from concourse.bass_utils import run_bass_kernel_spmd

    if "nc" not in _CACHE:
        _CACHE["nc"] = _build_bass()
    nc = _CACHE["nc"]

    in_maps = _prep_inputs(inputs)
    try:
        res = run_bass_kernel_spmd(nc, in_maps, core_ids=list(range(B)))
    except Exception:
        # Transient NRT_EXEC_UNIT_UNRECOVERABLE from a wedged device clears
        # on retry (observed twice on first run after idle).
        res = run_bass_kernel_spmd(nc, in_maps, core_ids=list(range(B)))
    _CACHE["last_result"] = res
    out = np.array([res.results[b]["out"][0, 0] for b in range(B)],
                   dtype=np.float32)
    return out



# revision 24
# speedup vs baseline: 2.1713x; 2.1713x over previous
"""Bass/Trainium2 kernel for nn_DTSP (GNN message passing, 8 graphs x K100).

Sharding: data-parallel, 1 graph per NeuronCore (8 cores). Each core runs the
full 32-step message-passing recurrence for its graph; the only cross-device
step is the host-side gather of the 8 per-graph vote scalars.

v2 (bf16 rewrite): fp32 matmuls cost 4 cycles/row on the TRN2 PE; bf16 costs
1. All tensors are bf16 in SBUF (PSUM accumulation stays fp32), which also
unlocks DVE 2x/4x packed modes for the LSTM elementwise ops.

Structural changes vs the fp32 baseline:
  - E-MLP layer 1 uses a block-anti-diagonal [128,128] stationary so one
    matmul processes both edge-halves (full PE array).
  - The edge->vertex path folds em_w2 AFTER aggregation:
      xV = em_w2^T (EV^T relu(L2)) + em_b2 (x) deg
    so the old 40-matmul msg layer disappears; L2 itself becomes the
    feature->edge-major layout flip (em_b1 is added via a rank-1 K=1 matmul
    into each flip PSUM tile since post-flip features sit on the free axis).
  - Gate activations batch 2560 edge-columns into 2 ACT instructions per
    gate ((i,f,o) sigmoid, g tanh) reading multi-bank PSUM tiles.
  - PSUM->SBUF evacuations are fused with bias+relu via DVE tensor_scalar
    where possible; gpsimd is avoided entirely (it locks the DVE SBUF port).

On-chip layout (per core): edges padded to 5120 and split in two halves of
2560 stacked on the partition axis (h0 on partitions 64:128 of state tiles,
h1 on 0:64; h1sb/h2sb use the opposite convention, matching the fp32
baseline). EV aggregation / scatter use host-prepared evr (edge-major
chunked) and evtx (transposed, with a row-degree row folding vm_b2).
"""

import os
import numpy as np

B = 8
V = 100
E = 4950
DIM = 64
NUM_MP = int(os.environ.get("DTSP_NUM_MP", "32"))
PHASE = int(os.environ.get("DTSP_PHASE", "99"))
VAR = int(os.environ.get("DTSP_VAR", "3"))
EPAD = 5120
H = EPAD // 2            # 2560 columns per half
CH = 512                 # psum chunk width (1 fp32 bank)
NCHUNK = H // CH         # 5
NSUB = H // 128          # 20 row-major 128-edge subchunks per half
NSUB_T = 2 * NSUB        # 40 subchunks total

_CACHE = {}


def _build_bass(num_mp=None, phase=None, var=None):
    global NUM_MP, PHASE, VAR
    if num_mp is not None:
        NUM_MP = num_mp
    if phase is not None:
        PHASE = phase
    if var is not None:
        VAR = var
    import concourse.bacc as bacc
    import concourse.tile as tile
    from concourse import mybir

    f32 = mybir.dt.float32
    bf16 = mybir.dt.bfloat16
    AF = mybir.ActivationFunctionType
    OP = mybir.AluOpType

    nc = bacc.Bacc("TRN2", target_bir_lowering=False, debug=False)

    # ---- DRAM I/O (bf16 except biases/out) ----
    d_wfc = nc.dram_tensor("wfc", [128, H], bf16, kind="ExternalInput")
    d_evr = nc.dram_tensor("evr", [128, NSUB_T * V], bf16, kind="ExternalInput")
    d_evtx = nc.dram_tensor("evtx", [V + 1, EPAD], bf16, kind="ExternalInput")
    d_wmlp = nc.dram_tensor("wmlp", [128, 6 * 64], bf16, kind="ExternalInput")
    d_wblk = nc.dram_tensor("wblk", [128, 3 * 128], bf16, kind="ExternalInput")
    d_wcat = nc.dram_tensor("wcat", [128, 3 * 256], bf16, kind="ExternalInput")
    d_winit = nc.dram_tensor("winit", [128, 120], bf16, kind="ExternalInput")
    d_bias = nc.dram_tensor("bias", [128, 24], f32, kind="ExternalInput")
    d_vmisc = nc.dram_tensor("vmisc", [128, 1024], bf16, kind="ExternalInput")
    d_vh0 = nc.dram_tensor("vh0", [128, 128], bf16, kind="ExternalInput")
    d_mlpx0 = nc.dram_tensor("mlpx0", [128, 64], bf16, kind="ExternalInput")
    d_out = nc.dram_tensor("out", [1, 1], f32, kind="ExternalOutput")

    with tile.TileContext(nc) as tc:
        import contextlib
        ctx = contextlib.ExitStack()
        with ctx:
            st = ctx.enter_context(tc.tile_pool(name="state", bufs=1))
            # big psum tiles: [128,1536]f32 = 3 banks; 2 bufs = 6 banks
            psb = ctx.enter_context(tc.tile_pool(name="psb", bufs=2, space="PSUM"))
            # aggregation accumulator: 1 bank
            psa = ctx.enter_context(tc.tile_pool(name="psa", bufs=1, space="PSUM"))
            # V-side psum: 1 bank
            psv = ctx.enter_context(tc.tile_pool(name="psv", bufs=1, space="PSUM"))

            # ---- persistent SBUF tiles ----
            evr = st.tile([128, NSUB_T * V], bf16, tag="evr")
            evtx = st.tile([V + 1, EPAD], bf16, tag="evtx")
            wmlp = st.tile([128, 6 * 64], bf16, tag="wmlp")
            wblk = st.tile([128, 3 * 128], bf16, tag="wblk")
            wcat = st.tile([128, 3 * 256], bf16, tag="wcat")
            winit = st.tile([128, 120], bf16, tag="winit")
            bias = st.tile([128, 24], f32, tag="bias")
            vmisc = st.tile([128, 1024], bf16, tag="vmisc")

            xh0 = st.tile([128, H], bf16, tag="xh0")   # [xE_h0 ; E_h0]
            xh1 = st.tile([128, H], bf16, tag="xh1")   # [E_h1 ; xE_h1]
            cE = st.tile([128, H], bf16, tag="cE")     # c state (h0 on 64:128)
            # L1 out, one tile per half; partition 64 = constant ones row so
            # the L2 flip matmul contracts K=65 and folds em_b1 in (also
            # forces every flip matmul to PE tile position (0,0) -- different
            # row-position matmuls into one PSUM bank fault the HW)
            h1sb0 = st.tile([65, H], bf16, tag="h10")
            h1sb1 = st.tile([65, H], bf16, tag="h11")
            h2sb = st.tile([128, H], bf16, tag="h2")   # vote L2 out
            h2e = st.tile([128, H], bf16, tag="h2e")   # edge-major relu(L2)
            s_i = st.tile([128, H], bf16, tag="si")
            s_f = st.tile([128, H], bf16, tag="sf")
            t_g = st.tile([128, H], bf16, tag="tg")
            s_o = st.tile([128, H], bf16, tag="so")    # gate-o dst == E state
            tcE = st.tile([128, H], bf16, tag="tc")

            # V-side small tiles
            xh_v = st.tile([128, 128], bf16, tag="xhv")   # [xV ; V]
            cV = st.tile([128, 128], bf16, tag="cv")
            sv = [st.tile([128, 128], bf16, tag=f"sv{g}", name=f"sv{g}") for g in range(4)]
            tcV = st.tile([128, 128], bf16, tag="tcv")
            h1v = st.tile([128, 128], bf16, tag="h1v")
            h2v = st.tile([128, 128], bf16, tag="h2v")
            mlpx = st.tile([128, 64], bf16, tag="mlpx")
            asb = st.tile([64, 128], bf16, tag="asb")     # aggregated h2 (A)
            rsum = st.tile([128, 1], f32, tag="rsum")
            rsumb = st.tile([128, 1], f32, tag="rsumb")
            outsb = st.tile([1, 4], f32, tag="outsb")

            # ---- weight views ----
            def wm(i):  # [128, 64] doubled weight i from wmlp
                return wmlp[:, i * 64:(i + 1) * 64]

            em_w0d, em_w2d = wm(0), wm(1)
            vm_w0d, vm_w1d, vm_w2d = wm(2), wm(3), wm(4)
            # wm(5) spare

            wblk_vt0 = wblk[:, 128:256]   # [[0,vt0],[vt0,0]]
            wblk_vt1 = wblk[:, 256:384]   # [[vt1,0],[0,vt1]]

            e_wcat0 = wcat[:, 0:256]
            e_wcat1 = wcat[:, 256:512]
            v_wcat = wcat[:, 512:768]

            iw0d = winit[:, 0:8]
            iw1d = winit[:, 8:24]
            iw2d = winit[:, 24:56]
            iw3d = winit[:, 56:120]

            # bias columns (f32; see host packing)
            def bcol(j):
                return bias[:, j:j + 1]

            ib0d, ib1d, ib2d, ib3d = bcol(0), bcol(1), bcol(2), bcol(3)
            em_b0d = bcol(4)
            vm_b0d, vm_b1d = bcol(6), bcol(7)
            vt_b0d, vt_b1d = bcol(8), bcol(9)
            be = [bcol(10 + g) for g in range(4)]     # E-LSTM gate biases
            bv = [bcol(14 + g) for g in range(4)]     # V-LSTM gate biases
            w2scaled = bcol(18)                        # [vt_w2;vt_w2]/E
            vt_b2d = bcol(19)

            emb2row = vmisc[0:1, 0:64]                 # em_b2 [1,64]
            degrow = vmisc[0:1, 64:64 + V]             # EV col-degrees [1,100]
            em_w1b = vmisc[0:65, 804:868]              # [em_w1 ; em_b1] [65,64]

            def cs(c):
                return slice(c * CH, (c + 1) * CH)

            GATE_FUNC = [AF.Sigmoid, AF.Sigmoid, AF.Tanh, AF.Sigmoid]
            GATE_DST = [s_i, s_f, t_g, s_o]
            # gate batching: chunk groups (col offset, width)
            GGRP = [(0, 1536), (1536, 1024)]

            # ================= INIT =================
            nc.sync.dma_start(out=winit[:], in_=d_winit[:])
            nc.sync.dma_start(out=bias[:], in_=d_bias[:])
            nc.sync.dma_start(out=vmisc[:], in_=d_vmisc[:])
            nc.sync.dma_start(out=wmlp[:], in_=d_wmlp[:])
            nc.sync.dma_start(out=wblk[:], in_=d_wblk[:])
            nc.sync.dma_start(out=wcat[:], in_=d_wcat[:])
            nc.sync.dma_start(out=xh_v[:], in_=d_vh0[:])
            nc.sync.dma_start(out=mlpx[:], in_=d_mlpx0[:])
            nc.sync.dma_start(out=evr[:], in_=d_evr[:])
            nc.sync.dma_start(out=evtx[:], in_=d_evtx[:])

            nc.vector.memset(cE[:], 0.0)
            nc.vector.memset(cV[:], 0.0)
            nc.vector.memset(h1sb0[64:65, :], 1.0)
            nc.vector.memset(h1sb1[64:65, :], 1.0)

            # init 4-layer MLP on [W,C] -> E0 (into s_o, then xh0/xh1)
            wfc = st.tile([128, H], bf16, tag="wfc")
            nc.sync.dma_start(out=wfc[:], in_=d_wfc[:])
            li1, li2, li3 = s_i, h2sb, tcE
            for c in range(NCHUNK):
                p = psb.tile([128, 1536], f32, tag="big")
                nc.tensor.matmul(p[0:8, 0:CH], iw0d[0:2, :], wfc[0:2, cs(c)])
                nc.tensor.matmul(p[32:40, 0:CH], iw0d[64:66, :], wfc[64:66, cs(c)])
                nc.scalar.activation(li1[0:8, cs(c)], p[0:8, 0:CH], AF.Relu, bias=ib0d[0:8])
                nc.scalar.activation(li1[32:40, cs(c)], p[32:40, 0:CH], AF.Relu, bias=ib0d[32:40])
            for c in range(NCHUNK):
                p = psb.tile([128, 1536], f32, tag="big")
                nc.tensor.matmul(p[0:16, 0:CH], iw1d[0:8, :], li1[0:8, cs(c)])
                nc.tensor.matmul(p[32:48, 0:CH], iw1d[32:40, :], li1[32:40, cs(c)])
                nc.scalar.activation(li2[0:16, cs(c)], p[0:16, 0:CH], AF.Relu, bias=ib1d[0:16])
                nc.scalar.activation(li2[32:48, cs(c)], p[32:48, 0:CH], AF.Relu, bias=ib1d[32:48])
            for c in range(NCHUNK):
                p = psb.tile([128, 1536], f32, tag="big")
                nc.tensor.matmul(p[0:32, 0:CH], iw2d[0:16, :], li2[0:16, cs(c)])
                nc.tensor.matmul(p[32:64, 0:CH], iw2d[32:48, :], li2[32:48, cs(c)])
                nc.scalar.activation(li3[0:32, cs(c)], p[0:32, 0:CH], AF.Relu, bias=ib2d[0:32])
                nc.scalar.activation(li3[32:64, cs(c)], p[32:64, 0:CH], AF.Relu, bias=ib2d[32:64])
            for c in range(NCHUNK):
                p = psb.tile([128, 1536], f32, tag="big")
                nc.tensor.matmul(p[64:128, 0:CH], iw3d[0:32, :], li3[0:32, cs(c)])
                nc.tensor.matmul(p[0:64, 0:CH], iw3d[32:64, :], li3[32:64, cs(c)])
                nc.scalar.activation(s_o[64:128, cs(c)], p[64:128, 0:CH], AF.Identity, bias=ib3d[64:128])
                nc.scalar.activation(s_o[0:64, cs(c)], p[0:64, 0:CH], AF.Identity, bias=ib3d[0:64])
            nc.vector.tensor_copy(xh0[64:128, :], s_o[64:128, :])
            nc.vector.tensor_copy(xh1[0:64, :], s_o[0:64, :])

            # ================= MP STEPS =================
            def emit_step():
                # --- E-MLP layer 1: h1 = relu(W0^T E + b0); per-half matmuls
                # on disjoint PE quadrants into different banks, both landing
                # on partitions 0:64 so the evacs stay offset-aligned ---
                for c in range(NCHUNK):
                    p = psb.tile([128, 1536], f32, tag="big")
                    nc.tensor.matmul(p[0:64, 0:CH], em_w0d[64:128, :], s_o[64:128, cs(c)])
                    nc.tensor.matmul(p[0:64, CH:2 * CH], em_w0d[0:64, :], s_o[0:64, cs(c)])
                    nc.vector.tensor_scalar(h1sb0[0:64, cs(c)], p[0:64, 0:CH],
                                            em_b0d[0:64], 0.0, OP.add, OP.max)
                    nc.vector.tensor_scalar(h1sb1[0:64, cs(c)], p[0:64, CH:2 * CH],
                                            em_b0d[64:128], 0.0, OP.add, OP.max)
                if PHASE < 2:
                    return
                # --- E-MLP layer 2 == layout flip to edge-major; K=65
                # contraction row adds em_b1; relu on evac ---
                for half, src in ((0, h1sb0), (1, h1sb1)):
                    for (i0, nsb) in ((0, 8), (8, 8), (16, 4)):
                        mp = psb.tile([128, 1536], f32, tag="big")
                        for k in range(nsb):
                            i = i0 + k
                            nc.tensor.matmul(
                                mp[:, k * 64:(k + 1) * 64],
                                src[0:65, i * 128:(i + 1) * 128],
                                em_w1b,
                            )
                        m0 = half * NSUB + i0
                        nc.vector.tensor_scalar(
                            h2e[:, m0 * 64:(m0 + nsb) * 64],
                            mp[:, 0:nsb * 64], 0.0, 0.0, OP.max, OP.add)
                if PHASE < 3:
                    return
                # --- aggregation: A = sum_m h2e_m^T @ evr_m  [64h, 100v] ---
                aggp = psa.tile([64, 512], f32, tag="acc")
                for m in range(NSUB_T):
                    nc.tensor.matmul(
                        aggp[:, 0:V],
                        h2e[:, m * 64:(m + 1) * 64],
                        evr[:, m * V:(m + 1) * V],
                        start=(m == 0),
                        stop=(m == NSUB_T - 1),
                    )
                nc.vector.tensor_copy(asb[:, 0:V], aggp[:, 0:V])
                # --- xV = em_w2^T A + em_b2 (x) deg ---
                vp = psv.tile([128, 512], f32, tag="v")
                nc.tensor.matmul(vp[0:64, 0:V], em_w2d[0:64, :], asb[0:64, 0:V],
                                 start=True, stop=False)
                nc.tensor.matmul(vp[0:64, 0:V], emb2row, degrow,
                                 start=False, stop=True)
                nc.scalar.activation(xh_v[0:64, 0:V], vp[0:64, 0:V], AF.Copy)
                if PHASE < 4:
                    return
                # --- V-LSTM ---
                for g in range(4):
                    vp = psv.tile([128, 512], f32, tag="v")
                    nc.tensor.matmul(vp[64:128, 0:V], v_wcat[:, g * 64:(g + 1) * 64],
                                     xh_v[:, 0:V])
                    nc.scalar.activation(sv[g][64:128, 0:V], vp[64:128, 0:V],
                                         GATE_FUNC[g], bias=bv[g][64:128])
                nc.vector.tensor_tensor(cV[64:128, 0:V], cV[64:128, 0:V], sv[1][64:128, 0:V], OP.mult)
                nc.vector.tensor_tensor(sv[2][64:128, 0:V], sv[0][64:128, 0:V], sv[2][64:128, 0:V], OP.mult)
                nc.vector.tensor_tensor(cV[64:128, 0:V], cV[64:128, 0:V], sv[2][64:128, 0:V], OP.add)
                nc.scalar.activation(tcV[64:128, 0:V], cV[64:128, 0:V], AF.Tanh)
                nc.vector.tensor_tensor(xh_v[64:128, 0:V], sv[3][64:128, 0:V], tcV[64:128, 0:V], OP.mult)
                # --- mlpV ---
                vp = psv.tile([128, 512], f32, tag="v")
                nc.tensor.matmul(vp[0:64, 0:V], vm_w0d[64:128, :], xh_v[64:128, 0:V])
                nc.scalar.activation(h1v[0:64, 0:V], vp[0:64, 0:V], AF.Relu, bias=vm_b0d[0:64])
                vp = psv.tile([128, 512], f32, tag="v")
                nc.tensor.matmul(vp[0:64, 0:V], vm_w1d[0:64, :], h1v[0:64, 0:V])
                nc.scalar.activation(h2v[0:64, 0:V], vp[0:64, 0:V], AF.Relu, bias=vm_b1d[0:64])
                vp = psv.tile([128, 512], f32, tag="v")
                nc.tensor.matmul(vp[0:V, 0:64], h2v[0:64, 0:V], vm_w2d[0:64, :])
                nc.scalar.activation(mlpx[0:V, :], vp[0:V, 0:64], AF.Copy)
                if PHASE < 5:
                    return
                # --- xE = mlpx^T @ EVT (+ rowdeg*vm_b2 via row 100) ---
                for (c0, w) in ((0, 1024), (1024, 1024), (2048, 512)):
                    px = psb.tile([128, 1536], f32, tag="big")
                    for sp in range(0, w, CH):
                        nc.tensor.matmul(px[0:64, sp:sp + CH], mlpx[0:V + 1, :],
                                         evtx[0:V + 1, c0 + sp:c0 + sp + CH])
                        nc.tensor.matmul(px[64:128, sp:sp + CH], mlpx[0:V + 1, :],
                                         evtx[0:V + 1, H + c0 + sp:H + c0 + sp + CH])
                    nc.scalar.activation(xh0[0:64, c0:c0 + w], px[0:64, 0:w], AF.Copy)
                    nc.vector.tensor_copy(xh1[64:128, c0:c0 + w], px[64:128, 0:w])
                if PHASE < 6:
                    return
                # --- E-LSTM gates: batched psum + one act per (gate, group) ---
                for g in range(4):
                    for (c0, w) in GGRP:
                        gp = psb.tile([128, 1536], f32, tag="big")
                        for sp in range(0, w, CH):
                            nc.tensor.matmul(gp[64:128, sp:sp + CH],
                                             e_wcat0[:, g * 64:(g + 1) * 64],
                                             xh0[:, c0 + sp:c0 + sp + CH])
                            nc.tensor.matmul(gp[0:64, sp:sp + CH],
                                             e_wcat1[:, g * 64:(g + 1) * 64],
                                             xh1[:, c0 + sp:c0 + sp + CH])
                        nc.scalar.activation(GATE_DST[g][:, c0:c0 + w], gp[:, 0:w],
                                             GATE_FUNC[g], bias=be[g])
                if PHASE < 7:
                    return
                # --- E-LSTM state update (bf16 DVE, 2 col groups) ---
                for (c0, w) in GGRP:
                    sl = slice(c0, c0 + w)
                    nc.vector.tensor_tensor(t_g[:, sl], s_i[:, sl], t_g[:, sl], OP.mult)
                    nc.vector.tensor_tensor(cE[:, sl], cE[:, sl], s_f[:, sl], OP.mult)
                    nc.vector.tensor_tensor(cE[:, sl], cE[:, sl], t_g[:, sl], OP.add)
                    nc.scalar.activation(tcE[:, sl], cE[:, sl], AF.Tanh)
                    nc.vector.tensor_tensor(s_o[:, sl], s_o[:, sl], tcE[:, sl], OP.mult)
                    nc.vector.tensor_copy(xh0[64:128, sl], s_o[64:128, sl])
                    nc.vector.tensor_copy(xh1[0:64, sl], s_o[0:64, sl])

            for _t in range(NUM_MP):
                emit_step()

            # ================= VOTE =================
            for c in range(NCHUNK):
                p = psb.tile([128, 1536], f32, tag="big")
                nc.tensor.matmul(p[:, 0:CH], wblk_vt0, s_o[:, cs(c)])
                nc.vector.tensor_scalar(s_i[:, cs(c)], p[:, 0:CH], vt_b0d, 0.0, OP.add, OP.max)
            for c in range(NCHUNK):
                p = psb.tile([128, 1536], f32, tag="big")
                nc.tensor.matmul(p[:, 0:CH], wblk_vt1, s_i[:, cs(c)])
                nc.vector.tensor_scalar(h2sb[:, cs(c)], p[:, 0:CH], vt_b1d, 0.0, OP.add, OP.max)
            # half0 = first 2560 padded edges (all real); half1 = 2390 real + pad
            nc.vector.reduce_sum(rsum[0:64, 0:1], h2sb[0:64, 0:H], axis=mybir.AxisListType.X)
            nc.vector.reduce_sum(rsum[64:128, 0:1], h2sb[64:128, 0:E - H], axis=mybir.AxisListType.X)
            nc.vector.tensor_copy(rsumb[:, 0:1], rsum[:, 0:1])
            vfin = psv.tile([128, 512], f32, tag="v")
            nc.tensor.matmul(vfin[0:1, 0:1], rsumb[:, 0:1], w2scaled[:, 0:1])
            nc.scalar.activation(outsb[0:1, 0:1], vfin[0:1, 0:1], AF.Identity, bias=vt_b2d[0:1])
            nc.sync.dma_start(out=d_out[:], in_=outsb[0:1, 0:1])

    nc.compile()
    return nc


def _prep_inputs(inputs):
    """Host-side: shard per graph + pack weights into the kernel's layouts."""
    import ml_dtypes
    bf = ml_dtypes.bfloat16
    gi = lambda k: np.asarray(inputs[k], dtype=np.float32)
    EV = gi("EV")
    Wfeat = gi("Wfeat").reshape(-1)
    C = gi("C").reshape(-1)

    # weights (shared across cores)
    def dbl(w):  # [64,64] -> [128,64] stacked twice
        return np.concatenate([w, w], axis=0)

    wmlp = np.concatenate(
        [dbl(gi("em_w0")), dbl(gi("em_w2")),
         dbl(gi("vm_w0")), dbl(gi("vm_w1")), dbl(gi("vm_w2")),
         np.zeros((128, 64), np.float32)], axis=1).astype(bf)     # [128, 384]

    def blk_anti(w):  # [[0,w],[w,0]]
        z = np.zeros((128, 128), np.float32)
        z[64:128, 0:64] = w
        z[0:64, 64:128] = w
        return z

    def blk_diag(w):  # [[w,0],[0,w]]
        z = np.zeros((128, 128), np.float32)
        z[0:64, 0:64] = w
        z[64:128, 64:128] = w
        return z

    wblk = np.concatenate(
        [blk_anti(gi("em_w0")), blk_anti(gi("vt_w0")),
         blk_diag(gi("vt_w1"))], axis=1).astype(bf)               # [128, 384]

    wih_e, whh_e = gi("wih_e"), gi("whh_e")
    wih_v, whh_v = gi("wih_v"), gi("whh_v")
    e_wcat0 = np.concatenate([wih_e, whh_e], axis=0)              # [128, 256]
    e_wcat1 = np.concatenate([whh_e, wih_e], axis=0)
    v_wcat = np.concatenate([wih_v, whh_v], axis=0)
    wcat = np.concatenate([e_wcat0, e_wcat1, v_wcat], axis=1).astype(bf)

    winit = np.zeros((128, 120), np.float32)
    w0, w1, w2, w3 = gi("init_w0"), gi("init_w1"), gi("init_w2"), gi("init_w3")
    winit[0:2, 0:8] = w0; winit[64:66, 0:8] = w0
    winit[0:8, 8:24] = w1; winit[32:40, 8:24] = w1
    winit[0:16, 24:56] = w2; winit[32:48, 24:56] = w2
    winit[0:32, 56:120] = w3; winit[32:64, 56:120] = w3
    winit = winit.astype(bf)

    bias = np.zeros((128, 24), np.float32)
    b0, b1, b2, b3 = gi("init_b0"), gi("init_b1"), gi("init_b2"), gi("init_b3")
    bias[0:8, 0] = b0; bias[32:40, 0] = b0
    bias[0:16, 1] = b1; bias[32:48, 1] = b1
    bias[0:32, 2] = b2; bias[32:64, 2] = b2
    bias[:, 3] = np.tile(b3, 2)
    bias[:, 4] = np.tile(gi("em_b0"), 2)
    bias[0:64, 6] = gi("vm_b0")
    bias[0:64, 7] = gi("vm_b1")
    bias[:, 8] = np.tile(gi("vt_b0"), 2)
    bias[:, 9] = np.tile(gi("vt_b1"), 2)
    bih_e, bhh_e = gi("bih_e"), gi("bhh_e")
    bih_v, bhh_v = gi("bih_v"), gi("bhh_v")
    for g in range(4):
        bias[:, 10 + g] = np.tile((bih_e + bhh_e)[g * 64:(g + 1) * 64], 2)
        bias[64:128, 14 + g] = (bih_v + bhh_v)[g * 64:(g + 1) * 64]
    vt_w2 = gi("vt_w2").reshape(-1)
    bias[:, 18] = np.tile(vt_w2, 2) / float(E)
    bias[0, 19] = float(gi("vt_b2").reshape(-1)[0])

    vh0 = np.zeros((128, 128), np.float32)
    vinit = gi("v_init").reshape(-1) / np.sqrt(np.float32(DIM))
    vh0[64:128, 0:V] = np.repeat(vinit[:, None], V, axis=1)
    vh0 = vh0.astype(bf)

    mlpx0 = np.zeros((128, 64), np.float32)
    mlpx0[V, :] = gi("vm_b2")
    mlpx0 = mlpx0.astype(bf)

    per_core = []
    for b in range(B):
        ev = EV[b * E:(b + 1) * E, b * V:(b + 1) * V]
        evp = np.zeros((EPAD, V), np.float32)
        evp[:E, :] = ev
        evrc = np.ascontiguousarray(
            evp.reshape(NSUB_T, 128, V).transpose(1, 0, 2).reshape(128, NSUB_T * V)
        ).astype(bf)
        evtxc = np.zeros((V + 1, EPAD), np.float32)
        evtxc[0:V, :] = evp.T
        evtxc[V, :] = evp.sum(axis=1)                               # row degrees
        evtxc = evtxc.astype(bf)
        degv = ev.sum(axis=0)                                       # col degrees
        vmisc = np.zeros((128, 1024), np.float32)
        vmisc[0, 0:64] = gi("em_b2")
        vmisc[0, 64:64 + V] = degv
        vmisc[0:64, 804:868] = gi("em_w1")
        vmisc[64, 804:868] = gi("em_b1")
        vmisc = vmisc.astype(bf)
        wfc = np.zeros((128, H), np.float32)
        w_b = Wfeat[b * E:(b + 1) * E]
        c_b = C[b * E:(b + 1) * E]
        wpad = np.zeros(EPAD, np.float32); wpad[:E] = w_b
        cpad = np.zeros(EPAD, np.float32); cpad[:E] = c_b
        wfc[0, :] = wpad[:H]; wfc[1, :] = cpad[:H]
        wfc[64, :] = wpad[H:]; wfc[65, :] = cpad[H:]
        wfc = wfc.astype(bf)
        per_core.append({
            "wfc": wfc, "evr": evrc, "evtx": evtxc, "wmlp": wmlp,
            "wblk": wblk, "wcat": wcat, "winit": winit, "bias": bias,
            "vmisc": vmisc, "vh0": vh0, "mlpx0": mlpx0,
        })
    return per_core


def kernel(**inputs):
    from concourse.bass_utils import run_bass_kernel_spmd

    if "nc" not in _CACHE:
        _CACHE["nc"] = _build_bass()
    nc = _CACHE["nc"]

    in_maps = _prep_inputs(inputs)
    try:
        res = run_bass_kernel_spmd(nc, in_maps, core_ids=list(range(B)))
    except Exception:
        # Transient NRT_EXEC_UNIT_UNRECOVERABLE from a wedged device clears
        # on retry (observed twice on first run after idle).
        res = run_bass_kernel_spmd(nc, in_maps, core_ids=list(range(B)))
    _CACHE["last_result"] = res
    out = np.array([res.results[b]["out"][0, 0] for b in range(B)],
                   dtype=np.float32)
    return out


# revision 27
# speedup vs baseline: 2.4692x; 1.1372x over previous
"""Bass/Trainium2 kernel for nn_DTSP (GNN message passing, 8 graphs x K100).

Sharding: data-parallel, 1 graph per NeuronCore (8 cores). Each core runs the
full 32-step message-passing recurrence for its graph; the only cross-device
step is the host-side gather of the 8 per-graph vote scalars.

v2 (bf16 rewrite): fp32 matmuls cost 4 cycles/row on the TRN2 PE; bf16 costs
1. All tensors are bf16 in SBUF (PSUM accumulation stays fp32), which also
unlocks DVE 2x/4x packed modes for the LSTM elementwise ops.

Structural changes vs the fp32 baseline:
  - E-MLP layer 1 uses a block-anti-diagonal [128,128] stationary so one
    matmul processes both edge-halves (full PE array).
  - The edge->vertex path folds em_w2 AFTER aggregation:
      xV = em_w2^T (EV^T relu(L2)) + em_b2 (x) deg
    so the old 40-matmul msg layer disappears; L2 itself becomes the
    feature->edge-major layout flip (em_b1 is added via a rank-1 K=1 matmul
    into each flip PSUM tile since post-flip features sit on the free axis).
  - Gate activations batch 2560 edge-columns into 2 ACT instructions per
    gate ((i,f,o) sigmoid, g tanh) reading multi-bank PSUM tiles.
  - PSUM->SBUF evacuations are fused with bias+relu via DVE tensor_scalar
    where possible; gpsimd is avoided entirely (it locks the DVE SBUF port).

On-chip layout (per core): edges padded to 5120 and split in two halves of
2560 stacked on the partition axis (h0 on partitions 64:128 of state tiles,
h1 on 0:64; h1sb/h2sb use the opposite convention, matching the fp32
baseline). EV aggregation / scatter use host-prepared evr (edge-major
chunked) and evtx (transposed, with a row-degree row folding vm_b2).
"""

import os
import numpy as np

B = 8
V = 100
E = 4950
DIM = 64
NUM_MP = int(os.environ.get("DTSP_NUM_MP", "32"))
PHASE = int(os.environ.get("DTSP_PHASE", "99"))
VAR = int(os.environ.get("DTSP_VAR", "3"))
EPAD = 5120
H = EPAD // 2            # 2560 columns per half
CH = 512                 # psum chunk width (1 fp32 bank)
NCHUNK = H // CH         # 5
NSUB = H // 128          # 20 row-major 128-edge subchunks per half
NSUB_T = 2 * NSUB        # 40 subchunks total

_CACHE = {}


def _build_bass(num_mp=None, phase=None, var=None):
    global NUM_MP, PHASE, VAR
    if num_mp is not None:
        NUM_MP = num_mp
    if phase is not None:
        PHASE = phase
    if var is not None:
        VAR = var
    import concourse.bacc as bacc
    import concourse.tile as tile
    from concourse import mybir

    f32 = mybir.dt.float32
    bf16 = mybir.dt.bfloat16
    AF = mybir.ActivationFunctionType
    OP = mybir.AluOpType

    nc = bacc.Bacc("TRN2", target_bir_lowering=False, debug=False)

    # ---- DRAM I/O (bf16 except biases/out) ----
    d_wfc = nc.dram_tensor("wfc", [128, H], bf16, kind="ExternalInput")
    d_evr = nc.dram_tensor("evr", [128, NSUB_T * V], bf16, kind="ExternalInput")
    d_evtx = nc.dram_tensor("evtx", [V + 1, EPAD], bf16, kind="ExternalInput")
    d_wmlp = nc.dram_tensor("wmlp", [128, 6 * 64], bf16, kind="ExternalInput")
    d_wblk = nc.dram_tensor("wblk", [128, 3 * 128], bf16, kind="ExternalInput")
    d_wcat = nc.dram_tensor("wcat", [128, 3 * 256], bf16, kind="ExternalInput")
    d_winit = nc.dram_tensor("winit", [128, 120], bf16, kind="ExternalInput")
    d_bias = nc.dram_tensor("bias", [128, 24], f32, kind="ExternalInput")
    d_vmisc = nc.dram_tensor("vmisc", [128, 1024], bf16, kind="ExternalInput")
    d_vh0 = nc.dram_tensor("vh0", [128, 128], bf16, kind="ExternalInput")
    d_mlpx0 = nc.dram_tensor("mlpx0", [128, 64], bf16, kind="ExternalInput")
    d_out = nc.dram_tensor("out", [1, 1], f32, kind="ExternalOutput")

    with tile.TileContext(nc) as tc:
        import contextlib
        ctx = contextlib.ExitStack()
        with ctx:
            st = ctx.enter_context(tc.tile_pool(name="state", bufs=1))
            # big psum tiles: [128,1536]f32 = 3 banks; 2 bufs = 6 banks
            psb = ctx.enter_context(tc.tile_pool(name="psb", bufs=2, space="PSUM"))
            # aggregation accumulator: 1 bank
            psa = ctx.enter_context(tc.tile_pool(name="psa", bufs=1, space="PSUM"))

            # ---- persistent SBUF tiles ----
            evr = st.tile([128, NSUB_T * V], bf16, tag="evr")
            evtx = st.tile([V + 1, EPAD], bf16, tag="evtx")
            wmlp = st.tile([128, 6 * 64], bf16, tag="wmlp")
            wblk = st.tile([128, 3 * 128], bf16, tag="wblk")
            wcat = st.tile([128, 3 * 256], bf16, tag="wcat")
            winit = st.tile([128, 120], bf16, tag="winit")
            bias = st.tile([128, 24], f32, tag="bias")
            vmisc = st.tile([128, 1024], bf16, tag="vmisc")

            xh0 = st.tile([128, H], bf16, tag="xh0")   # [xE_h0 ; E_h0]
            xh1 = st.tile([128, H], bf16, tag="xh1")   # [E_h1 ; xE_h1]
            cE = st.tile([128, H], bf16, tag="cE")     # c state (h0 on 64:128)
            # L1 out, one tile per half; partition 64 = constant ones row so
            # the L2 flip matmul contracts K=65 and folds em_b1 in (also
            # forces every flip matmul to PE tile position (0,0) -- different
            # row-position matmuls into one PSUM bank fault the HW)
            h1sb0 = st.tile([65, H], bf16, tag="h10")
            h1sb1 = st.tile([65, H], bf16, tag="h11")
            h2sb = st.tile([128, H], bf16, tag="h2")   # vote L2 out
            h2e = st.tile([128, H], bf16, tag="h2e")   # edge-major relu(L2)
            s_i = st.tile([128, H], bf16, tag="si")
            s_f = st.tile([128, H], bf16, tag="sf")
            t_g = st.tile([128, H], bf16, tag="tg")
            s_o = st.tile([128, H], bf16, tag="so")    # gate-o dst == E state
            tcE = st.tile([128, H], bf16, tag="tc")

            # V-side small tiles
            xh_v = st.tile([128, 128], bf16, tag="xhv")   # [xV ; V]
            cV = st.tile([128, 128], bf16, tag="cv")
            sv = [st.tile([128, 128], bf16, tag=f"sv{g}", name=f"sv{g}") for g in range(4)]
            tcV = st.tile([128, 128], bf16, tag="tcv")
            h1v = st.tile([128, 128], bf16, tag="h1v")
            h2v = st.tile([128, 128], bf16, tag="h2v")
            mlpx = st.tile([128, 64], bf16, tag="mlpx")
            asb = st.tile([64, 128], bf16, tag="asb")     # aggregated h2 (A)
            rsum = st.tile([128, 1], f32, tag="rsum")
            rsumb = st.tile([128, 1], f32, tag="rsumb")
            outsb = st.tile([1, 4], f32, tag="outsb")

            # ---- weight views ----
            def wm(i):  # [128, 64] doubled weight i from wmlp
                return wmlp[:, i * 64:(i + 1) * 64]

            em_w0d, em_w2d = wm(0), wm(1)
            vm_w0d, vm_w1d, vm_w2d = wm(2), wm(3), wm(4)
            # wm(5) spare

            wblk_vt0 = wblk[:, 128:256]   # [[0,vt0],[vt0,0]]
            wblk_vt1 = wblk[:, 256:384]   # [[vt1,0],[0,vt1]]

            e_wcat0 = wcat[:, 0:256]
            e_wcat1 = wcat[:, 256:512]
            v_wcat = wcat[:, 512:768]

            iw0d = winit[:, 0:8]
            iw1d = winit[:, 8:24]
            iw2d = winit[:, 24:56]
            iw3d = winit[:, 56:120]

            # bias columns (f32; see host packing)
            def bcol(j):
                return bias[:, j:j + 1]

            ib0d, ib1d, ib2d, ib3d = bcol(0), bcol(1), bcol(2), bcol(3)
            em_b0d = bcol(4)
            vm_b0d, vm_b1d = bcol(6), bcol(7)
            vt_b0d, vt_b1d = bcol(8), bcol(9)
            be = [bcol(10 + g) for g in range(4)]     # E-LSTM gate biases
            bv = [bcol(14 + g) for g in range(4)]     # V-LSTM gate biases
            w2scaled = bcol(18)                        # [vt_w2;vt_w2]/E
            vt_b2d = bcol(19)

            emb2row = vmisc[0:1, 0:64]                 # em_b2 [1,64]
            degrow = vmisc[0:1, 64:64 + V]             # EV col-degrees [1,100]
            em_w1b = vmisc[0:65, 804:868]              # [em_w1 ; em_b1] [65,64]

            def cs(c):
                return slice(c * CH, (c + 1) * CH)

            GATE_FUNC = [AF.Sigmoid, AF.Sigmoid, AF.Tanh, AF.Sigmoid]
            GATE_DST = [s_i, s_f, t_g, s_o]
            # gate batching: chunk groups (col offset, width)
            GGRP = [(0, 1536), (1536, 1024)]

            # ================= INIT =================
            nc.sync.dma_start(out=winit[:], in_=d_winit[:])
            nc.sync.dma_start(out=bias[:], in_=d_bias[:])
            nc.sync.dma_start(out=vmisc[:], in_=d_vmisc[:])
            nc.sync.dma_start(out=wmlp[:], in_=d_wmlp[:])
            nc.sync.dma_start(out=wblk[:], in_=d_wblk[:])
            nc.sync.dma_start(out=wcat[:], in_=d_wcat[:])
            nc.sync.dma_start(out=xh_v[:], in_=d_vh0[:])
            nc.sync.dma_start(out=mlpx[:], in_=d_mlpx0[:])
            nc.sync.dma_start(out=evr[:], in_=d_evr[:])
            nc.sync.dma_start(out=evtx[:], in_=d_evtx[:])

            nc.vector.memset(cE[:], 0.0)
            nc.vector.memset(cV[:], 0.0)
            nc.vector.memset(h1sb0[64:65, :], 1.0)
            nc.vector.memset(h1sb1[64:65, :], 1.0)

            # init 4-layer MLP on [W,C] -> E0 (into s_o, then xh0/xh1)
            wfc = st.tile([128, H], bf16, tag="wfc")
            nc.sync.dma_start(out=wfc[:], in_=d_wfc[:])
            li1, li2, li3 = s_i, h2sb, tcE
            for c in range(NCHUNK):
                p = psb.tile([128, 1536], f32, tag="big")
                nc.tensor.matmul(p[0:8, 0:CH], iw0d[0:2, :], wfc[0:2, cs(c)])
                nc.tensor.matmul(p[32:40, 0:CH], iw0d[64:66, :], wfc[64:66, cs(c)])
                nc.scalar.activation(li1[0:8, cs(c)], p[0:8, 0:CH], AF.Relu, bias=ib0d[0:8])
                nc.scalar.activation(li1[32:40, cs(c)], p[32:40, 0:CH], AF.Relu, bias=ib0d[32:40])
            for c in range(NCHUNK):
                p = psb.tile([128, 1536], f32, tag="big")
                nc.tensor.matmul(p[0:16, 0:CH], iw1d[0:8, :], li1[0:8, cs(c)])
                nc.tensor.matmul(p[32:48, 0:CH], iw1d[32:40, :], li1[32:40, cs(c)])
                nc.scalar.activation(li2[0:16, cs(c)], p[0:16, 0:CH], AF.Relu, bias=ib1d[0:16])
                nc.scalar.activation(li2[32:48, cs(c)], p[32:48, 0:CH], AF.Relu, bias=ib1d[32:48])
            for c in range(NCHUNK):
                p = psb.tile([128, 1536], f32, tag="big")
                nc.tensor.matmul(p[0:32, 0:CH], iw2d[0:16, :], li2[0:16, cs(c)])
                nc.tensor.matmul(p[32:64, 0:CH], iw2d[32:48, :], li2[32:48, cs(c)])
                nc.scalar.activation(li3[0:32, cs(c)], p[0:32, 0:CH], AF.Relu, bias=ib2d[0:32])
                nc.scalar.activation(li3[32:64, cs(c)], p[32:64, 0:CH], AF.Relu, bias=ib2d[32:64])
            for c in range(NCHUNK):
                p = psb.tile([128, 1536], f32, tag="big")
                nc.tensor.matmul(p[64:128, 0:CH], iw3d[0:32, :], li3[0:32, cs(c)])
                nc.tensor.matmul(p[0:64, 0:CH], iw3d[32:64, :], li3[32:64, cs(c)])
                nc.scalar.activation(s_o[64:128, cs(c)], p[64:128, 0:CH], AF.Identity, bias=ib3d[64:128])
                nc.scalar.activation(s_o[0:64, cs(c)], p[0:64, 0:CH], AF.Identity, bias=ib3d[0:64])
            nc.vector.tensor_copy(xh0[64:128, :], s_o[64:128, :])
            nc.vector.tensor_copy(xh1[0:64, :], s_o[0:64, :])

            # ================= MP STEPS =================
            def emit_step():
                # --- E-MLP layer 1: h1 = relu(W0^T E + b0); per-half matmuls
                # on disjoint PE quadrants into per-half tiles (different
                # banks -> concurrent-safe); evacs batched [64, 1024] with h0
                # on DVE and h1 on ACT to balance the engines ---
                for (c0, w) in ((0, 1024), (1024, 1024), (2048, 512)):
                    p0 = psb.tile([128, 1536], f32, tag="big")
                    p1 = psb.tile([128, 1536], f32, tag="big")
                    for sp in range(0, w, CH):
                        nc.tensor.matmul(p0[0:64, sp:sp + CH], em_w0d[64:128, :],
                                         s_o[64:128, c0 + sp:c0 + sp + CH])
                        nc.tensor.matmul(p1[0:64, sp:sp + CH], em_w0d[0:64, :],
                                         s_o[0:64, c0 + sp:c0 + sp + CH])
                    nc.vector.tensor_scalar(h1sb0[0:64, c0:c0 + w], p0[0:64, 0:w],
                                            em_b0d[0:64], 0.0, OP.add, OP.max)
                    nc.scalar.activation(h1sb1[0:64, c0:c0 + w], p1[0:64, 0:w],
                                         AF.Relu, bias=em_b0d[0:64])
                if PHASE < 2:
                    return
                # --- E-MLP layer 2 == layout flip to edge-major; K=65
                # contraction row adds em_b1; relu on evac ---
                for half, src in ((0, h1sb0), (1, h1sb1)):
                    for (i0, nsb) in ((0, 8), (8, 8), (16, 4)):
                        mp = psb.tile([128, 1536], f32, tag="big")
                        for k in range(nsb):
                            i = i0 + k
                            nc.tensor.matmul(
                                mp[:, k * 64:(k + 1) * 64],
                                src[0:65, i * 128:(i + 1) * 128],
                                em_w1b,
                            )
                        m0 = half * NSUB + i0
                        nc.vector.tensor_scalar(
                            h2e[:, m0 * 64:(m0 + nsb) * 64],
                            mp[:, 0:nsb * 64], 0.0, 0.0, OP.max, OP.add)
                if PHASE < 3:
                    return
                # --- aggregation: A = sum_m h2e_m^T @ evr_m  [64h, 100v] ---
                aggp = psa.tile([64, 512], f32, tag="acc")
                for m in range(NSUB_T):
                    nc.tensor.matmul(
                        aggp[:, 0:V],
                        h2e[:, m * 64:(m + 1) * 64],
                        evr[:, m * V:(m + 1) * V],
                        start=(m == 0),
                        stop=(m == NSUB_T - 1),
                    )
                nc.vector.tensor_copy(asb[:, 0:V], aggp[:, 0:V])
                # --- xV = em_w2^T A + em_b2 (x) deg ---
                vp = psb.tile([128, 1536], f32, tag="big")
                nc.tensor.matmul(vp[0:64, 0:V], em_w2d[0:64, :], asb[0:64, 0:V],
                                 start=True, stop=False)
                nc.tensor.matmul(vp[0:64, 0:V], emb2row, degrow,
                                 start=False, stop=True)
                nc.scalar.activation(xh_v[0:64, 0:V], vp[0:64, 0:V], AF.Copy)
                if PHASE < 4:
                    return
                # --- V-LSTM ---
                for g in range(4):
                    vp = psb.tile([128, 1536], f32, tag="big")
                    nc.tensor.matmul(vp[64:128, 0:V], v_wcat[:, g * 64:(g + 1) * 64],
                                     xh_v[:, 0:V])
                    nc.scalar.activation(sv[g][64:128, 0:V], vp[64:128, 0:V],
                                         GATE_FUNC[g], bias=bv[g][64:128])
                nc.vector.tensor_tensor(cV[64:128, 0:V], cV[64:128, 0:V], sv[1][64:128, 0:V], OP.mult)
                nc.vector.tensor_tensor(sv[2][64:128, 0:V], sv[0][64:128, 0:V], sv[2][64:128, 0:V], OP.mult)
                nc.vector.tensor_tensor(cV[64:128, 0:V], cV[64:128, 0:V], sv[2][64:128, 0:V], OP.add)
                nc.scalar.activation(tcV[64:128, 0:V], cV[64:128, 0:V], AF.Tanh)
                nc.vector.tensor_tensor(xh_v[64:128, 0:V], sv[3][64:128, 0:V], tcV[64:128, 0:V], OP.mult)
                # --- mlpV ---
                vp = psb.tile([128, 1536], f32, tag="big")
                nc.tensor.matmul(vp[0:64, 0:V], vm_w0d[64:128, :], xh_v[64:128, 0:V])
                nc.scalar.activation(h1v[0:64, 0:V], vp[0:64, 0:V], AF.Relu, bias=vm_b0d[0:64])
                vp = psb.tile([128, 1536], f32, tag="big")
                nc.tensor.matmul(vp[0:64, 0:V], vm_w1d[0:64, :], h1v[0:64, 0:V])
                nc.scalar.activation(h2v[0:64, 0:V], vp[0:64, 0:V], AF.Relu, bias=vm_b1d[0:64])
                vp = psb.tile([128, 1536], f32, tag="big")
                nc.tensor.matmul(vp[0:V, 0:64], h2v[0:64, 0:V], vm_w2d[0:64, :])
                nc.scalar.activation(mlpx[0:V, :], vp[0:V, 0:64], AF.Copy)
                if PHASE < 5:
                    return
                # --- xE = mlpx^T @ EVT (+ rowdeg*vm_b2 via row 100) ---
                for (c0, w) in ((0, 1024), (1024, 1024), (2048, 512)):
                    px = psb.tile([128, 1536], f32, tag="big")
                    for sp in range(0, w, CH):
                        nc.tensor.matmul(px[0:64, sp:sp + CH], mlpx[0:V + 1, :],
                                         evtx[0:V + 1, c0 + sp:c0 + sp + CH])
                        nc.tensor.matmul(px[64:128, sp:sp + CH], mlpx[0:V + 1, :],
                                         evtx[0:V + 1, H + c0 + sp:H + c0 + sp + CH])
                    nc.scalar.activation(xh0[0:64, c0:c0 + w], px[0:64, 0:w], AF.Copy)
                    nc.vector.tensor_copy(xh1[64:128, c0:c0 + w], px[64:128, 0:w])
                if PHASE < 6:
                    return
                # --- E-LSTM gates: batched psum + one act per (gate, group) ---
                for (c0, w) in GGRP:
                    for g in range(4):
                        gp = psb.tile([128, 1536], f32, tag="big")
                        for sp in range(0, w, CH):
                            nc.tensor.matmul(gp[64:128, sp:sp + CH],
                                             e_wcat0[:, g * 64:(g + 1) * 64],
                                             xh0[:, c0 + sp:c0 + sp + CH])
                            nc.tensor.matmul(gp[0:64, sp:sp + CH],
                                             e_wcat1[:, g * 64:(g + 1) * 64],
                                             xh1[:, c0 + sp:c0 + sp + CH])
                        nc.scalar.activation(GATE_DST[g][:, c0:c0 + w], gp[:, 0:w],
                                             GATE_FUNC[g], bias=be[g])
                if PHASE < 7:
                    return
                # --- E-LSTM state update (bf16 DVE, 2 col groups) ---
                for (c0, w) in GGRP:
                    sl = slice(c0, c0 + w)
                    nc.vector.tensor_tensor(t_g[:, sl], s_i[:, sl], t_g[:, sl], OP.mult)
                    nc.vector.tensor_tensor(cE[:, sl], cE[:, sl], s_f[:, sl], OP.mult)
                    nc.vector.tensor_tensor(cE[:, sl], cE[:, sl], t_g[:, sl], OP.add)
                    nc.scalar.activation(tcE[:, sl], cE[:, sl], AF.Tanh)
                    nc.vector.tensor_tensor(s_o[:, sl], s_o[:, sl], tcE[:, sl], OP.mult)
                    nc.vector.tensor_copy(xh0[64:128, sl], s_o[64:128, sl])
                    nc.vector.tensor_copy(xh1[0:64, sl], s_o[0:64, sl])

            for _t in range(NUM_MP):
                emit_step()

            # ================= VOTE =================
            for c in range(NCHUNK):
                p = psb.tile([128, 1536], f32, tag="big")
                nc.tensor.matmul(p[:, 0:CH], wblk_vt0, s_o[:, cs(c)])
                nc.vector.tensor_scalar(s_i[:, cs(c)], p[:, 0:CH], vt_b0d, 0.0, OP.add, OP.max)
            for c in range(NCHUNK):
                p = psb.tile([128, 1536], f32, tag="big")
                nc.tensor.matmul(p[:, 0:CH], wblk_vt1, s_i[:, cs(c)])
                nc.vector.tensor_scalar(h2sb[:, cs(c)], p[:, 0:CH], vt_b1d, 0.0, OP.add, OP.max)
            # half0 = first 2560 padded edges (all real); half1 = 2390 real + pad
            nc.vector.reduce_sum(rsum[0:64, 0:1], h2sb[0:64, 0:H], axis=mybir.AxisListType.X)
            nc.vector.reduce_sum(rsum[64:128, 0:1], h2sb[64:128, 0:E - H], axis=mybir.AxisListType.X)
            nc.vector.tensor_copy(rsumb[:, 0:1], rsum[:, 0:1])
            vfin = psa.tile([64, 512], f32, tag="acc")
            nc.tensor.matmul(vfin[0:1, 0:1], rsumb[:, 0:1], w2scaled[:, 0:1])
            nc.scalar.activation(outsb[0:1, 0:1], vfin[0:1, 0:1], AF.Identity, bias=vt_b2d[0:1])
            nc.sync.dma_start(out=d_out[:], in_=outsb[0:1, 0:1])

    nc.compile()
    return nc


def _prep_inputs(inputs):
    """Host-side: shard per graph + pack weights into the kernel's layouts."""
    import ml_dtypes
    bf = ml_dtypes.bfloat16
    gi = lambda k: np.asarray(inputs[k], dtype=np.float32)
    EV = gi("EV")
    Wfeat = gi("Wfeat").reshape(-1)
    C = gi("C").reshape(-1)

    # weights (shared across cores)
    def dbl(w):  # [64,64] -> [128,64] stacked twice
        return np.concatenate([w, w], axis=0)

    wmlp = np.concatenate(
        [dbl(gi("em_w0")), dbl(gi("em_w2")),
         dbl(gi("vm_w0")), dbl(gi("vm_w1")), dbl(gi("vm_w2")),
         np.zeros((128, 64), np.float32)], axis=1).astype(bf)     # [128, 384]

    def blk_anti(w):  # [[0,w],[w,0]]
        z = np.zeros((128, 128), np.float32)
        z[64:128, 0:64] = w
        z[0:64, 64:128] = w
        return z

    def blk_diag(w):  # [[w,0],[0,w]]
        z = np.zeros((128, 128), np.float32)
        z[0:64, 0:64] = w
        z[64:128, 64:128] = w
        return z

    wblk = np.concatenate(
        [blk_anti(gi("em_w0")), blk_anti(gi("vt_w0")),
         blk_diag(gi("vt_w1"))], axis=1).astype(bf)               # [128, 384]

    wih_e, whh_e = gi("wih_e"), gi("whh_e")
    wih_v, whh_v = gi("wih_v"), gi("whh_v")
    e_wcat0 = np.concatenate([wih_e, whh_e], axis=0)              # [128, 256]
    e_wcat1 = np.concatenate([whh_e, wih_e], axis=0)
    v_wcat = np.concatenate([wih_v, whh_v], axis=0)
    wcat = np.concatenate([e_wcat0, e_wcat1, v_wcat], axis=1).astype(bf)

    winit = np.zeros((128, 120), np.float32)
    w0, w1, w2, w3 = gi("init_w0"), gi("init_w1"), gi("init_w2"), gi("init_w3")
    winit[0:2, 0:8] = w0; winit[64:66, 0:8] = w0
    winit[0:8, 8:24] = w1; winit[32:40, 8:24] = w1
    winit[0:16, 24:56] = w2; winit[32:48, 24:56] = w2
    winit[0:32, 56:120] = w3; winit[32:64, 56:120] = w3
    winit = winit.astype(bf)

    bias = np.zeros((128, 24), np.float32)
    b0, b1, b2, b3 = gi("init_b0"), gi("init_b1"), gi("init_b2"), gi("init_b3")
    bias[0:8, 0] = b0; bias[32:40, 0] = b0
    bias[0:16, 1] = b1; bias[32:48, 1] = b1
    bias[0:32, 2] = b2; bias[32:64, 2] = b2
    bias[:, 3] = np.tile(b3, 2)
    bias[:, 4] = np.tile(gi("em_b0"), 2)
    bias[0:64, 6] = gi("vm_b0")
    bias[0:64, 7] = gi("vm_b1")
    bias[:, 8] = np.tile(gi("vt_b0"), 2)
    bias[:, 9] = np.tile(gi("vt_b1"), 2)
    bih_e, bhh_e = gi("bih_e"), gi("bhh_e")
    bih_v, bhh_v = gi("bih_v"), gi("bhh_v")
    for g in range(4):
        bias[:, 10 + g] = np.tile((bih_e + bhh_e)[g * 64:(g + 1) * 64], 2)
        bias[64:128, 14 + g] = (bih_v + bhh_v)[g * 64:(g + 1) * 64]
    vt_w2 = gi("vt_w2").reshape(-1)
    bias[:, 18] = np.tile(vt_w2, 2) / float(E)
    bias[0, 19] = float(gi("vt_b2").reshape(-1)[0])

    vh0 = np.zeros((128, 128), np.float32)
    vinit = gi("v_init").reshape(-1) / np.sqrt(np.float32(DIM))
    vh0[64:128, 0:V] = np.repeat(vinit[:, None], V, axis=1)
    vh0 = vh0.astype(bf)

    mlpx0 = np.zeros((128, 64), np.float32)
    mlpx0[V, :] = gi("vm_b2")
    mlpx0 = mlpx0.astype(bf)

    per_core = []
    for b in range(B):
        ev = EV[b * E:(b + 1) * E, b * V:(b + 1) * V]
        evp = np.zeros((EPAD, V), np.float32)
        evp[:E, :] = ev
        evrc = np.ascontiguousarray(
            evp.reshape(NSUB_T, 128, V).transpose(1, 0, 2).reshape(128, NSUB_T * V)
        ).astype(bf)
        evtxc = np.zeros((V + 1, EPAD), np.float32)
        evtxc[0:V, :] = evp.T
        evtxc[V, :] = evp.sum(axis=1)                               # row degrees
        evtxc = evtxc.astype(bf)
        degv = ev.sum(axis=0)                                       # col degrees
        vmisc = np.zeros((128, 1024), np.float32)
        vmisc[0, 0:64] = gi("em_b2")
        vmisc[0, 64:64 + V] = degv
        vmisc[0:64, 804:868] = gi("em_w1")
        vmisc[64, 804:868] = gi("em_b1")
        vmisc = vmisc.astype(bf)
        wfc = np.zeros((128, H), np.float32)
        w_b = Wfeat[b * E:(b + 1) * E]
        c_b = C[b * E:(b + 1) * E]
        wpad = np.zeros(EPAD, np.float32); wpad[:E] = w_b
        cpad = np.zeros(EPAD, np.float32); cpad[:E] = c_b
        wfc[0, :] = wpad[:H]; wfc[1, :] = cpad[:H]
        wfc[64, :] = wpad[H:]; wfc[65, :] = cpad[H:]
        wfc = wfc.astype(bf)
        per_core.append({
            "wfc": wfc, "evr": evrc, "evtx": evtxc, "wmlp": wmlp,
            "wblk": wblk, "wcat": wcat, "winit": winit, "bias": bias,
            "vmisc": vmisc, "vh0": vh0, "mlpx0": mlpx0,
        })
    return per_core


def kernel(**inputs):
    from concourse.bass_utils import run_bass_kernel_spmd

    if "nc" not in _CACHE:
        _CACHE["nc"] = _build_bass()
    nc = _CACHE["nc"]

    in_maps = _prep_inputs(inputs)
    try:
        res = run_bass_kernel_spmd(nc, in_maps, core_ids=list(range(B)))
    except Exception:
        # Transient NRT_EXEC_UNIT_UNRECOVERABLE from a wedged device clears
        # on retry (observed twice on first run after idle).
        res = run_bass_kernel_spmd(nc, in_maps, core_ids=list(range(B)))
    _CACHE["last_result"] = res
    out = np.array([res.results[b]["out"][0, 0] for b in range(B)],
                   dtype=np.float32)
    return out
